# revision 26
# baseline (speedup 1.0000x reference)
"""Trainium2 Bass kernel for MultiHeadEdgeAwareMessagePassing.

Math restructure (validated vs reference):
  logits[i,j,h] = s_q[i,h] + s_k[j,h] + w[i,j]*c1[h] + c0[h]   (valid j: w>0)
  alpha = softmax_j(logits) * w
s_q, c0 cancel in the softmax; bk's contribution cancels too. With
g[j,h] = exp(h[j]@a_k[h]), a_k[h] = u_k[h] @ Wk[h-block], v = h@Wv^T (+bv):
  msg[i,h,:] = Num_h[i,:] / Den_h[i]
  Num_h = W1^T (g_h*v_h)  (+ (W1^T g_h) * bv, handled via Bo2 epilogue term)
  Den_h = mask^T g_h + c1_h (W1^T g_h)
where mask=[w>0], W1=relu(w)  (exp(c1 w) ~= 1 + c1 w).

Device-side structure (per core, 384 destination rows):
  - A short junk-matmul stream at t=0 keeps the PE busy so the HAM
    clock-gate opens (2.4 GHz) before real work arrives.
  - 12 blocks of 256 src nodes: fp8 DoubleRow matmuls contract both
    128-j tiles at once: one v|s_k projection per j-tile (fp8 h
    stationary, fp8 [64*WvT | 256*a_k] moving), exp(scale=1/256) on Act,
    relu->fp8 W1 on Act, mask on DVE, g-scale on DVE, then per i-subtile
    one DoubleRow big matmul (fp8 W1 stationary, fp8 [g*v | c1*g] moving)
    interleaved with den matmuls (g stationary - 4-col LDWEIGHTS, mask
    moving, [4, 384] accumulator).
  - Epilogue (stage-parallel across the 3 i-subtiles to keep the strict
    FIFO engines fed): den combine + reciprocal, msg, PE transposes, Wo
    projection + Bo2 bias term + residual via identity-matmul
    accumulation; mean arrives free as matmul column 256 (row-sum
    columns of WoT/hs/Bo2b), variance via Act Square with accum_out,
    rstd via Sqrt+reciprocal (Square and Sqrt share act-set 3, so the
    single mid-kernel table switch overlaps DVE work).

All small-weight algebra (a_k, c1, Wo^T/64 packing, h+bo residual,
row-sum columns) is host layout prep; all O(N^2)/O(N*D) work runs on
device. w is host-cast to bf16 and repacked partition-contiguous; h to
fp8 e4m3 (scales folded into Wv/a_k/Wo on host).
"""

import numpy as np

N = 3072
D = 256
H = 4
DH = 64
DE = 8
NCORES = 8
ISLICE = N // NCORES   # 384
NSUB = ISLICE // 128   # 3
NBLK = 12              # blocks of 2 j-tiles (256 src nodes)
NJUNK = 6              # HAM warm-up matmuls

_cache = {}


def _build_bass():
    import concourse.bass as bass
    import concourse.tile as tile
    from concourse import bacc, mybir
    from concourse.bass import ts

    dt = mybir.dt
    AF = mybir.ActivationFunctionType
    OP = mybir.AluOpType

    nc = bacc.Bacc("TRN2", target_bir_lowering=False, debug=False,
                   num_devices=NCORES)

    bf = dt.bfloat16
    f32 = dt.float32
    f8 = dt.float8e4

    wt_d = nc.dram_tensor("wt", [128, NBLK * 2 * ISLICE], bf,
                          kind="ExternalInput")
    ht_d = nc.dram_tensor("ht", [D, N], f8, kind="ExternalInput")
    wvak_d = nc.dram_tensor("wvak", [128, 2 * 272], f8, kind="ExternalInput")
    c1b_d = nc.dram_tensor("c1b", [128, H], bf, kind="ExternalInput")
    # suep bf16: WoT2 0:512 | ident 512:640 | gamma 640:896 | beta 896:1152
    #            | Bo2b rows 0:4 @ 1152:1408
    suep_d = nc.dram_tensor("suep", [128, 1424], bf, kind="ExternalInput")
    hs_d = nc.dram_tensor("hs", [ISLICE, D + 1], bf, kind="ExternalInput")
    out_d = nc.dram_tensor("out", [ISLICE, D], f32, kind="ExternalOutput")

    with tile.TileContext(nc) as tc:
        with (
            tc.tile_pool(name="consts", bufs=1) as consts,
            tc.tile_pool(name="wtp", bufs=6) as wtp,
            tc.tile_pool(name="elem", bufs=6) as elem,
            tc.tile_pool(name="rhsp", bufs=6) as rhsp,
            tc.tile_pool(name="gp", bufs=6) as gp,
            tc.tile_pool(name="small", bufs=4) as small,
            tc.tile_pool(name="outp", bufs=4) as outp,
            tc.tile_pool(name="acc", bufs=1, space="PSUM") as accp,
            tc.tile_pool(name="denp", bufs=1, space="PSUM") as denp,
            tc.tile_pool(name="vskp", bufs=4, space="PSUM") as vskp,
        ):
            # ---- PE warm-up: junk matmul stream so HAM unthrottles ----
            junk = consts.tile([128, 384], bf, tag="junk")
            nc.gpsimd.memset(junk, 0.0)
            eps_sb = consts.tile([128, 1], f32, tag="eps")
            nc.gpsimd.memset(eps_sb, 1e-5)
            s256_sb = consts.tile([128, 1], f32, tag="s256")
            nc.gpsimd.memset(s256_sb, 1.0 / 256.0)
            ps_junk = denp.tile([128, 384], f32, tag="den", name="psjunk")
            for _ in range(NJUNK):
                nc.tensor.matmul(ps_junk, junk[:, 0:128], junk,
                                 start=True, stop=True, skip_group_check=True)

            # ---- epilogue consts (DMA'd mid-loop, see blk==3) ----
            suep = consts.tile([128, 1424], bf, tag="suep")
            hs32 = consts.tile([128, NSUB, D + 1], bf, tag="hs32")
            WoT2 = suep[:, 0:514].rearrange("p (a n) -> p a n", a=2)
            ident = suep[:, 514:642]
            gam_sb = suep[:, 642:898]
            bet_sb = suep[:, 898:1154]
            bo2b = suep[0:4, 1154:1411]

            # ---- first wt block goes out first; then the small consts ----
            wt_first = wtp.tile([128, 2, ISLICE], bf, tag="wt", name="wt0")
            nc.sync.dma_start(
                wt_first, wt_d[:, ts(0, 2 * ISLICE)].rearrange(
                    "p (j i) -> p j i", j=2))
            wvak = consts.tile([128, 2, 272], f8, tag="wvak")
            nc.sync.dma_start(wvak, wvak_d.ap().rearrange(
                "p (a n) -> p a n", a=2))
            c1b = consts.tile([128, H], bf, tag="c1b")
            nc.sync.dma_start(c1b, c1b_d.ap())

            ht_sb = consts.tile([128, 2, N], f8, tag="ht")
            ht_re = ht_d.ap().rearrange("(a p) n -> p a n", p=128)

            # ---------------- persistent accumulators ----------------
            psA = [accp.tile([128, 260], f32, tag=f"A{s}", name=f"psA{s}")
                   for s in range(NSUB)]
            ps_den = denp.tile([4, 384], f32, tag="den", name="psden")

            # ------- main loop: 12 blocks of 256 src nodes ----------------
            for blk in range(NBLK):
                if blk % 2 == 0:
                    ch = blk // 2
                    nc.sync.dma_start(ht_sb[:, :, ts(ch, 512)],
                                      ht_re[:, :, ts(ch, 512)])
                if blk == 0:
                    wt2 = wt_first
                else:
                    wt2 = wtp.tile([128, 2, ISLICE], bf, tag="wt",
                                   name=f"wt{blk}")
                    nc.sync.dma_start(
                        wt2, wt_d[:, ts(blk, 2 * ISLICE)].rearrange(
                            "p (j i) -> p j i", j=2))
                if blk == 2:
                    # epilogue consts mid-stream, spread far apart to keep
                    # the wt-stream blips small
                    nc.sync.dma_start(suep, suep_d.ap())
                if blk == 7:
                    nc.sync.dma_start(
                        hs32, hs_d.ap().rearrange("(s p) n -> p s n", p=128))

                W1b = elem.tile([128, 2, ISLICE], f8, tag="W1",
                                name=f"W1_{blk}")
                nc.scalar.activation(W1b, wt2, AF.Relu)
                mskb = elem.tile([128, 2, ISLICE], bf, tag="msk",
                                 name=f"msk{blk}")
                nc.vector.tensor_scalar(mskb, wt2, 0.0, None, op0=OP.is_gt)

                # fused v|s_k projection: one DoubleRow matmul per j-tile
                g32 = gp.tile([128, 2, H], bf, tag="g32", name=f"g{blk}")
                rhs4 = rhsp.tile([128, 2, 272], f8, tag="rhs",
                                 name=f"rhs{blk}")
                c1bb = bass.AP(tensor=c1b.tensor, offset=c1b.offset,
                               ap=[c1b.ap[0], [0, 2], c1b.ap[1]])
                for jj in range(2):
                    jt = blk * 2 + jj
                    vskj = vskp.tile([128, 260], f32, tag="vsk",
                                     name=f"vsk{blk}_{jj}")
                    nc.tensor.matmul(vskj,
                                     ht_sb[:, :, ts(jt, 128)],
                                     wvak[:, :, 0:260],
                                     start=True, stop=True,
                                     perf_mode=mybir.MatmulPerfMode.DoubleRow,
                                     skip_group_check=True)
                    nc.scalar.activation(g32[:, jj, :], vskj[:, 256:260],
                                         AF.Exp, scale=s256_sb)
                    gj = g32[:, jj, :]
                    g32b = bass.AP(tensor=gj.tensor, offset=gj.offset,
                                   ap=[gj.ap[0], gj.ap[1], [0, DH]])
                    nc.vector.tensor_tensor(
                        out=rhs4[:, jj, 0:256].rearrange(
                            "p (h d) -> p h d", h=H),
                        in0=vskj[:, 0:256].rearrange(
                            "p (h d) -> p h d", h=H),
                        in1=g32b, op=OP.mult)
                nc.vector.tensor_tensor(out=rhs4[:, :, 256:260], in0=g32,
                                        in1=c1bb, op=OP.mult)

                # big DoubleRow matmuls interleaved with den matmuls so the
                # 256-col LDWEIGHTS hides behind neighbouring streams
                st = (blk == 0)
                sp = (blk == NBLK - 1)
                for s in range(NSUB):
                    if s < 2:
                        nc.tensor.matmul(ps_den, g32[:, s, :],
                                         mskb[:, s, :],
                                         start=(st and s == 0),
                                         stop=(sp and s == 1),
                                         skip_group_check=True)
                    nc.tensor.matmul(psA[s], W1b[:, :, ts(s, 128)],
                                     rhs4[:, :, 0:260],
                                     start=st, stop=sp,
                                     perf_mode=mybir.MatmulPerfMode.DoubleRow,
                                     skip_group_check=True)

            # hoist the sqrt act-table load off the epilogue critical path
            dumt = small.tile([1, 1], f32, tag="dum")
            nc.scalar.activation(dumt, eps_sb[0:1, :], AF.Sqrt)

            # ------- epilogue: stage-parallel across the 3 i-subtiles -----
            den4 = consts.tile([4, 384], bf, tag="den4")
            nc.scalar.activation(den4, ps_den, AF.Copy)

            ps_t4s, denTs = [], []
            for s in range(NSUB):
                ps_t4 = vskp.tile([128, H], bf, tag="vsk", name=f"pst4_{s}")
                nc.tensor.transpose(ps_t4, den4[:, ts(s, 128)],
                                    ident[0:4, 0:4])
                ps_t4s.append(ps_t4)
            for s in range(NSUB):
                denT = small.tile([128, H], bf, tag="denT", name=f"dT{s}")
                nc.scalar.activation(denT, ps_t4s[s], AF.Copy)
                denTs.append(denT)

            dens, rdens = [], []
            for s in range(NSUB):
                den = small.tile([128, H], f32, tag="den", name=f"den{s}")
                nc.vector.tensor_tensor(out=den, in0=psA[s][:, 256:260],
                                        in1=denTs[s], op=OP.add)
                dens.append(den)
            for s in range(NSUB):
                rden = small.tile([128, H], f32, tag="rden", name=f"rden{s}")
                nc.vector.reciprocal(rden, dens[s])
                rdens.append(rden)

            msgs, grdens = [], []
            for s in range(NSUB):
                msg = outp.tile([128, D], bf, tag="msg", name=f"msg{s}")
                rdb = bass.AP(tensor=rdens[s].tensor, offset=rdens[s].offset,
                              ap=[rdens[s].ap[0], rdens[s].ap[1], [0, DH]])
                nc.vector.tensor_tensor(
                    out=msg.rearrange("p (h d) -> p h d", h=H),
                    in0=psA[s][:, 0:256].rearrange("p (h d) -> p h d", h=H),
                    in1=rdb, op=OP.mult)
                msgs.append(msg)
                grden = small.tile([128, H], bf, tag="grden", name=f"gr{s}")
                nc.vector.tensor_tensor(out=grden, in0=psA[s][:, 256:260],
                                        in1=rdens[s], op=OP.mult)
                grdens.append(grden)

            ps_ts, ps_gs = [], []
            for s in range(NSUB):
                ps_t = vskp.tile([128, 2, 128], bf, tag="vsk",
                                 name=f"pst{s}")
                for b in range(2):
                    nc.tensor.transpose(ps_t[:, b, :], msgs[s][:, ts(b, 128)],
                                        ident)
                ps_ts.append(ps_t)
                ps_g = vskp.tile([4, 128], bf, tag="vsk", name=f"psg{s}")
                nc.tensor.transpose(ps_g, grdens[s], ident)
                ps_gs.append(ps_g)

            msgTs, grdTs = [], []
            for s in range(NSUB):
                msgT = outp.tile([128, 2, 128], bf, tag="msgT",
                                 name=f"msgT{s}")
                nc.scalar.activation(msgT, ps_ts[s], AF.Copy)
                msgTs.append(msgT)
                grdT = small.tile([4, 128], bf, tag="grdT", name=f"gT{s}")
                nc.scalar.activation(grdT, ps_gs[s], AF.Copy)
                grdTs.append(grdT)

            ps_os = []
            for s in range(NSUB):
                ps_o = accp.tile([128, D + 1], f32, tag=f"A{s}",
                                 name=f"pso{s}")
                nc.tensor.matmul(ps_o, msgTs[s][:, 0, :], WoT2[:, 0, :],
                                 start=True, stop=False,
                                 skip_group_check=True)
                nc.tensor.matmul(ps_o, msgTs[s][:, 1, :], WoT2[:, 1, :],
                                 start=False, stop=False,
                                 skip_group_check=True)
                nc.tensor.matmul(ps_o, grdTs[s], bo2b, start=False,
                                 stop=False, skip_group_check=True)
                nc.tensor.matmul(ps_o, ident, hs32[:, s, :], start=False,
                                 stop=True, skip_group_check=True)
                ps_os.append(ps_o)

            mus, ssqs = [], []
            for s in range(NSUB):
                mu = small.tile([128, 1], f32, tag="mu", name=f"mu{s}")
                nc.scalar.activation(mu, ps_os[s][:, 256:257], AF.Copy,
                                     scale=s256_sb)
                mus.append(mu)
                xsq = outp.tile([128, D], f32, tag="xsq", name=f"xq{s}")
                ssq = small.tile([128, 1], f32, tag="ssq", name=f"sq{s}")
                nc.scalar.activation(xsq, ps_os[s][:, 0:256], AF.Square,
                                     accum_out=ssq)
                ssqs.append(ssq)

            ys = []
            for s in range(NSUB):
                y = outp.tile([128, D], f32, tag="y", name=f"y{s}")
                nc.vector.scalar_tensor_tensor(
                    out=y, in0=ps_os[s][:, 0:256], scalar=mus[s], in1=gam_sb,
                    op0=OP.subtract, op1=OP.mult)
                ys.append(y)

            sds = []
            for s in range(NSUB):
                mu2 = small.tile([128, 1], f32, tag="mu2", name=f"m2{s}")
                nc.vector.tensor_tensor(out=mu2, in0=mus[s], in1=mus[s],
                                        op=OP.mult)
                var = small.tile([128, 1], f32, tag="var", name=f"va{s}")
                nc.vector.scalar_tensor_tensor(
                    out=var, in0=ssqs[s], scalar=s256_sb, in1=mu2,
                    op0=OP.mult, op1=OP.subtract)
                sd = small.tile([128, 1], f32, tag="sd", name=f"sd{s}")
                nc.scalar.activation(sd, var, AF.Sqrt, bias=eps_sb)
                sds.append(sd)

            for s in range(NSUB):
                rstd = small.tile([128, 1], f32, tag="rstd", name=f"rs{s}")
                nc.vector.reciprocal(rstd, sds[s])
                yg = outp.tile([128, D], f32, tag="yg", name=f"yg{s}")
                nc.vector.scalar_tensor_tensor(
                    out=yg, in0=ys[s], scalar=rstd, in1=bet_sb,
                    op0=OP.mult, op1=OP.add)
                nc.sync.dma_start(out_d[ts(s, 128), :], yg)

    nc.compile()
    return nc


def _make_in_maps(h, w, Wq, bq, Wk, bk, Wv, bv, We_w, We_b, u, Wo, bo,
                  gamma, beta, **_unused):
    import ml_dtypes
    f = np.float32
    b16 = ml_dtypes.bfloat16
    f8 = ml_dtypes.float8_e4m3

    h = np.ascontiguousarray(h, dtype=f)
    wT = np.ascontiguousarray(np.asarray(w, dtype=f).T)
    Wk = np.asarray(Wk, dtype=f)
    Wv = np.asarray(Wv, dtype=f)
    Wo = np.asarray(Wo, dtype=f)
    u = np.asarray(u, dtype=f)
    We_w = np.asarray(We_w, dtype=f)
    bv = np.asarray(bv, dtype=f)
    bo = np.asarray(bo, dtype=f)

    # host-side small-weight algebra
    u_k = u[:, DH:2 * DH]
    u_e = u[:, 2 * DH:2 * DH + DE]
    a_k = np.stack([u_k[hh] @ Wk[hh * DH:(hh + 1) * DH, :]
                    for hh in range(H)])                       # [H, 256]
    c1 = np.array([We_w[hh * DE:(hh + 1) * DE, 0] @ u_e[hh]
                   for hh in range(H)], dtype=f)               # [H]

    # wvak fp8 [128, 2*260]: per d-half a: 64*WvT | 256*a_k^T
    WvT = Wv.T
    wvak = np.zeros((128, 2, 272), f)
    for a in range(2):
        wvak[:, a, 0:256] = 64.0 * WvT[a * 128:(a + 1) * 128, :]
        wvak[:, a, 256:260] = 256.0 * a_k[:, a * 128:(a + 1) * 128].T
    wvak = wvak.reshape(128, 544)

    c1b = np.broadcast_to(c1, (128, H)).copy()

    # suep bf16: WoT2 | ident | gamma | beta | Bo2b
    WoT2 = Wo.T / 64.0
    b = np.float32
    WoT2q = WoT2.astype(b16).astype(f)
    # blocks of 257 cols: [WoT2 block | row-sum col] so x's row sum (-> mu)
    # falls out of the projection matmul as column 256
    wo_blk = np.zeros((2, 128, D + 1), f)
    for a in range(2):
        wo_blk[a, :, 0:256] = WoT2q[a * 128:(a + 1) * 128, :]
        wo_blk[a, :, 256] = WoT2q[a * 128:(a + 1) * 128, :].sum(axis=1)
    suep = np.zeros((128, 1424), f)
    suep[:, 0:514] = wo_blk.transpose(1, 0, 2).reshape(128, 514)
    suep[:, 514:642] = np.eye(128, dtype=f)
    suep[:, 642:898] = np.asarray(gamma, dtype=f)[None, :]
    suep[:, 898:1154] = np.asarray(beta, dtype=f)[None, :]
    # Bo2b[h] = 64*bv_h @ WoT2_h-block / c1_h  (bias term via Gw*c1*rden)
    c1_safe = np.where(np.abs(c1) < 1e-30, 1.0, c1)
    bo2 = np.zeros((H, D + 1), f)
    for hh in range(H):
        bo2[hh, 0:256] = (64.0 * bv[hh * DH:(hh + 1) * DH]
                          @ WoT2q[hh * DH:(hh + 1) * DH, :]) / c1_safe[hh]
        bo2[hh, 256] = bo2[hh, 0:256].sum()
    suep[0:4, 1154:1411] = bo2

    common = {
        "ht": np.ascontiguousarray(h.T.astype(f8)),
        "wvak": wvak.astype(f8),
        "c1b": c1b.astype(b16),
        "suep": suep.astype(b16),
    }
    in_maps = []
    for c in range(NCORES):
        sl = slice(c * ISLICE, (c + 1) * ISLICE)
        m = dict(common)
        wts = wT[:, sl].reshape(NBLK, 2, 128, ISLICE).transpose(2, 0, 1, 3)
        m["wt"] = np.ascontiguousarray(wts.reshape(128, NBLK * 2 * ISLICE)
                                       .astype(b16))
        hsq = (h[sl, :] + bo[None, :]).astype(b16).astype(f)
        hsx = np.concatenate([hsq, hsq.sum(axis=1, keepdims=True)], axis=1)
        m["hs"] = np.ascontiguousarray(hsx.astype(b16))
        in_maps.append(m)
    return in_maps


def kernel(**inputs):
    from concourse.bass_utils import run_bass_kernel_spmd

    if "nc" not in _cache:
        _cache["nc"] = _build_bass()
    nc = _cache["nc"]

    in_maps = _make_in_maps(**inputs)
    res = run_bass_kernel_spmd(nc, in_maps, core_ids=list(range(NCORES)))
    out = np.concatenate([r["out"] for r in res.results], axis=0)
    return np.ascontiguousarray(out, dtype=np.float32)
